# revision 18
# baseline (speedup 1.0000x reference)
"""Trainium2 kernel for nn_CP1_17669495456474 (sparse_attention).
8-core data-parallel: core = (sample, spatial half). Device computes the
grouped cross-correlation (out[l, p] = sum_k W[k, l] * F[k, p], K=1024,
L=1024, P=2016 per core) on the fp32r tensor engine; host applies the
cheap elementwise fuse/mask/softmax.

Per-matmul: stationary W = Bl[:, chi, j, lb, :] -> [K=128, M=128] l-block
(contiguous, as the BIR verifier requires 1 free dim for weights); moving
F = Fa[:, chi, 8*pb:8*pb+8, j:j+63] -> [K=128, N=504] (8 h-rows x 63 w).
K=128 partitions pack (4 patch-row offsets i) x (32 channels); the patch
column offset j lives purely in the free-dim access patterns. 8 accumulation
steps (4 j x 2 chi) complete the K=1024 contraction in one PSUM bank.

Host staging (per core, half = top/bottom 32 h-rows of the 63x63 grid):
  fR[g=chi*4+i, c, r, w]     = fpad[32*chi+c, rowbase+i+r, w]
  bL[g=chi*4+i, c, j, lb, u] = bnpad[32*chi+c, 2*lh+i, 2*lw+j],
                               l = 128*lb+u, lh = l//32, lw = l%32
(compacted: only the stride-2 rows a given i-offset ever reads are shipped).
"""
import sys, types
import numpy as np
from numpy.lib.stride_tricks import sliding_window_view

import concourse.bass as bass
import concourse.mybir as mybir
from concourse.tile import TileContext
import concourse.tile as tile_mod
import concourse.bass_utils as bass_utils

F32 = mybir.dt.float32
F32R = mybir.dt.float32r
L = 1024
NLB, NPB, PB = 8, 4, 512   # l-blocks of 128, p-blocks of 512 (8 h-rows x 64-wide, col 63 garbage)

# ---------------- compile workarounds (walrus sync-wait limits) ----------------
import orjson

def _patched_drain_and_barrier(self, tick_clock, wait_clock):
    nc = self.nc
    ScopedClock = tile_mod.ScopedClock
    drain_inst = nc.sync.drain()
    wait_clock.add_sem_waits(drain_inst.ins, ScopedClock({None: tick_clock.global_clock}))
    waits = list(drain_inst.ins.sync_info.on_wait)
    if len(waits) > 1:
        import bass_rust
        drain_inst.ins.sync_info = bass_rust.SyncInfo(on_wait=waits[:1], on_update=[])
        for i in range(1, len(waits)):
            d2 = nc.sync.drain()
            d2.ins.sync_info = bass_rust.SyncInfo(on_wait=[waits[i]], on_update=[])
    nc.all_engine_barrier()
    popped = nc._tile_sem_poison_stack.pop()
    assert popped is self._sem_poison
    nc.clear_and_free_semaphores(list(self.sems.allocated().values()))
    nc.all_engine_barrier()

def _strip_mm_updates(m):
    """Drop per-Matmult PE-clock sem increments except those that are wait
    targets (group-final MMs). MMs complete in pc order, so an inc on the
    last MM of a group covers the whole group. All wait values on that sem
    are remapped to the kept-inc numbering."""
    for f in m.get("functions", []):
        insts = [i for b in f.get("blocks", []) for i in b.get("instructions", [])]
        # the PE clock sem = the sem Matmult instructions inc
        mm_sems = set()
        for inst in insts:
            if inst.get("opcode") == "Matmult":
                for u in (inst.get("sync_info") or {}).get("on_update") or []:
                    if u.get("update_mode") == "sem-inc":
                        mm_sems.add(u["id"])
        if len(mm_sems) != 1:
            continue
        sem = mm_sems.pop()
        # collect waited-on values for this sem
        targets = set()
        for inst in insts:
            for w in (inst.get("sync_info") or {}).get("on_wait") or []:
                if w.get("id") == sem:
                    targets.add(w["wait_value"])
        # decide keep/remove per inc, build orig->kept prefix map
        count = 0
        kept = 0
        prefix = {0: 0}
        for inst in insts:
            si = inst.get("sync_info") or {}
            ups = si.get("on_update") or []
            for u in ups:
                if u.get("id") == sem and u.get("update_mode") == "sem-inc":
                    v = u.get("update_value", 1)
                    removable = (inst.get("opcode") == "Matmult"
                                 and not inst.get("stop_tensor_calc")
                                 and v == 1
                                 and (count + 1) not in targets)
                    count += v
                    if removable:
                        si["on_update"] = [x for x in ups if x is not u]
                    else:
                        kept += v
                    prefix[count] = kept
        # remap waits
        for inst in insts:
            for w in (inst.get("sync_info") or {}).get("on_wait") or []:
                if w.get("id") == sem:
                    w["wait_value"] = prefix.get(w["wait_value"], kept)
    return m

def _split_waits_json(bir_bytes):
    m = _strip_mm_updates(orjson.loads(bir_bytes))
    for f in m.get("functions", []):
        for b in f.get("blocks", []):
            insts = b.get("instructions", [])
            out = []
            for inst in insts:
                si = inst.get("sync_info")
                waits = (si or {}).get("on_wait") or []
                opc = inst.get("opcode", "")
                is_dma = opc.startswith("DMA") or "Trigger" in opc or "Dma" in opc
                keep = 1
                if is_dma and len(waits) <= 1:
                    out.append(inst)
                    continue
                if len(waits) > keep:
                    si["on_wait"] = waits[-keep:]
                    for i, w in enumerate(waits[:-keep]):
                        out.append({
                            "debug": inst.get("debug", 0), "engine": inst["engine"],
                            "ins": [], "outs": [], "name": f"{inst['name']}_xw{i}",
                            "opcode": "EventSemaphore",
                            "sync_info": {"on_update": [], "on_wait": [w]},
                        })
                out.append(inst)
            b["instructions"] = out
    return orjson.dumps(m)

def _install_patches():
    if getattr(bass_utils.compile_bir_kernel, "_wait_split", False):
        return
    TileContext._drain_and_barrier = _patched_drain_and_barrier
    import concourse.bass2jax as b2j
    orig = bass_utils.compile_bir_kernel
    def wrapped(bir_str, *a, **kw):
        if isinstance(bir_str, (bytes, bytearray)):
            try:
                bir_str = _split_waits_json(bir_str)
            except Exception:
                pass
        return orig(bir_str, *a, **kw)
    wrapped._wait_split = True
    bass_utils.compile_bir_kernel = wrapped
    if hasattr(b2j, "compile_bir_kernel"):
        b2j.compile_bir_kernel = wrapped
    # NTFF hook shim so trace=True doesn't crash if requested elsewhere
    if "antenv.axon_hooks" not in sys.modules:
        mod = types.ModuleType("antenv.axon_hooks")
        mod._hook = None
        mod.set_axon_ntff_profile_hook = lambda h: setattr(mod, "_hook", h)
        mod.get_axon_ntff_profile_hook = lambda: mod._hook
        sys.modules["antenv.axon_hooks"] = mod
        try:
            from trn_agent_boot.trn_boot import _ntff_profile_via_ctypes
            hk = _ntff_profile_via_ctypes('/opt/axon/libaxon_pjrt.so')
            if hk is not None:
                mod._hook = hk
        except Exception:
            pass
        bass_utils.upload_artifacts = lambda tmpdir: str(tmpdir)

# ---------------- device program: raw cos in [l, p] tiles ----------------
_NC_CACHE = [None]

def _build_nc():
    if _NC_CACHE[0] is not None:
        return _NC_CACHE[0]
    _install_patches()
    nc = bass.Bass("TRN2", target_bir_lowering=False, debug=False)
    fR = nc.dram_tensor("fR", [2, 128, 32, 68], F32R, kind="ExternalInput")
    bL = nc.dram_tensor("bL", [2, 128, 4, NLB, 128], F32R, kind="ExternalInput")
    o_d = nc.dram_tensor("o", [NLB, 128, NPB * PB], F32, kind="ExternalOutput")
    with TileContext(nc) as tc:
        import contextlib
        ctx = contextlib.ExitStack()
        with ctx:
            const = ctx.enter_context(tc.tile_pool(name="const", bufs=1))
            outp = ctx.enter_context(tc.tile_pool(name="outp", bufs=4))
            psp = ctx.enter_context(tc.tile_pool(name="psp", bufs=4, space="PSUM"))
            Fa = const.tile([128, 2, 32, 68], F32R, tag="Fa")
            Bl = const.tile([128, 2, 4, NLB, 128], F32R, tag="Bl")
            # Large input DMAs (one per chi x chunk) on the two HWDGE rings,
            # issued in consumption order for the pb-major matmul loop: each
            # Bl lb-chunk unlocks a full 8-matmul group, so the PE never
            # starves after the first group.
            def _ld_bl(lbl, lbh):
                for chi in range(2):
                    eng = nc.sync if chi == 0 else nc.scalar
                    eng.dma_start(out=Bl[:, chi, :, lbl:lbh, :],
                                  in_=bL[chi, :, :, lbl:lbh, :])
            def _ld_fa(rl, rh):
                for chi in range(2):
                    eng = nc.sync if chi == 0 else nc.scalar
                    eng.dma_start(out=Fa[:, chi, rl:rh, :],
                                  in_=fR[chi, :, rl:rh, :])
            # PE warm-up: dummy matmuls on a zeroed tile while input DMAs
            # are in flight, so the HAM clock gate is released (K=8/8) before
            # the first real matmul issues.
            dums = const.tile([128, 512], F32, tag="dums")
            nc.vector.memset(dums[:], 0.0)
            psd = psp.tile([128, PB], F32, tag="psd", name="psd", bufs=1)
            for _wi in range(12):
                nc.tensor.matmul(psd[0:128, :], dums[:, 0:128].bitcast(F32R),
                                 dums[:, :].bitcast(F32R),
                                 start=True, stop=True, skip_group_check=True)
            _ld_bl(0, 1)
            _ld_fa(0, 12)
            _ld_bl(1, 2)
            _ld_bl(2, 3)
            _ld_bl(3, 4)
            _ld_bl(4, 5)
            _ld_bl(5, 6)
            _ld_bl(6, 7)
            _ld_bl(7, 8)
            _ld_fa(12, 22)
            _ld_fa(22, 32)
            cnt = 0
            for pb in range(NPB):
                for lb in range(NLB):
                    ps = psp.tile([128, PB], F32, tag="ps", name="ps")
                    kk = 0
                    for chi in range(2):
                        for j in range(4):
                            nc.tensor.matmul(ps[0:128, :],
                                             Bl[:, chi, j, lb, :],
                                             Fa[:, chi, 8*pb:8*pb+8, j:j+64],
                                             start=(kk == 0), stop=(kk == 7),
                                             skip_group_check=True)
                            kk += 1
                    O = outp.tile([128, PB], F32, tag="O", name="O")
                    last = (pb == NPB - 1 and lb == NLB - 1)
                    if last:
                        # drain the final tile on both engines/rings in parallel
                        nc.scalar.copy(out=O[0:128, 0:PB//2], in_=ps[0:128, 0:PB//2])
                        nc.vector.tensor_copy(O[0:128, PB//2:PB], ps[0:128, PB//2:PB])
                        nc.sync.dma_start(out=o_d[lb, :, PB*pb:PB*pb+PB//2],
                                          in_=O[0:128, 0:PB//2])
                        nc.scalar.dma_start(out=o_d[lb, :, PB*pb+PB//2:PB*pb+PB],
                                            in_=O[0:128, PB//2:PB])
                    else:
                        if cnt % 2 == 0:
                            nc.scalar.copy(out=O[0:128, :], in_=ps[0:128, :])
                        else:
                            nc.vector.tensor_copy(O[0:128, :], ps[0:128, :])
                        eng = nc.sync if cnt % 2 == 0 else nc.scalar
                        eng.dma_start(out=o_d[lb, :, PB*pb:PB*pb+PB], in_=O[0:128, :])
                    cnt += 1
    _NC_CACHE[0] = nc
    return nc

# ---------------- host side ----------------
def _pad_edge(x):
    return np.pad(x, ((0, 0), (1, 1), (1, 1)), mode='edge')

def _stage_core(fpad, bpad, half):
    rowbase = 32 * half
    fwin = np.zeros((64, 35, 68), np.float32)
    avail = min(35, 66 - rowbase)
    fwin[:, :avail, :66] = fpad[:, rowbase:rowbase + avail]
    fR = np.empty((2, 128, 32, 68), np.float32)
    bL = np.empty((2, 128, 4, NLB, 128), np.float32)
    for chi in range(2):
        for i in range(4):
            fR[chi, 32*i:32*i+32] = fwin[32*chi:32*chi+32, i:i+32, :]
            for j in range(4):
                sub = bpad[32*chi:32*chi+32, i:i+63:2, j:j+63:2]  # (32 c, 32 lh, 32 lw)
                bL[chi, 32*i:32*i+32, j] = sub.reshape(32, NLB, 128)
    return {"fR": fR, "bL": bL}

def _make_in_maps(f, b):
    f = np.asarray(f, dtype=np.float32)
    b = np.asarray(b, dtype=np.float32)
    bn = b / np.sqrt((b * b).sum(axis=(2, 3), keepdims=True) + 1e-8)
    in_maps = []
    for core in range(8):
        smp, half = core // 2, core % 2
        in_maps.append(_stage_core(_pad_edge(f[smp]), _pad_edge(bn[smp]), half))
    return in_maps

def _diag3(x):
    N, M = x.shape
    xp = np.zeros((N + 2, M + 2), x.dtype)
    xp[1:N+1, 1:M+1] = x
    return xp[0:N, 0:M] + xp[1:N+1, 1:M+1] + xp[2:N+2, 2:M+2]

def _epilogue(cos, maskc1):
    """cos (1024, 63, 63) raw; maskc1 (64, 64) = 1-mask -> softmax out."""
    cs, hs, ws = L, 63, 63
    c1 = _diag3(cos.reshape(cs, hs * ws))
    c1 = c1.reshape(32, 32, hs, ws).transpose(1, 0, 3, 2).reshape(cs, hs * ws)
    c1 = _diag3(c1)
    c1 = c1.reshape(32, 32, hs, ws).transpose(1, 0, 3, 2).reshape(cs, hs, ws)
    mcp = np.pad(maskc1, ((1, 1), (1, 1)), mode='edge')
    sw = sliding_window_view(mcp, (4, 4))          # (63, 63, 4, 4)
    mmk = sw[::2, ::2].mean(axis=(2, 3)).reshape(cs).astype(np.float32)
    mmp = sw.mean(axis=(2, 3)).astype(np.float32)  # (63, 63)
    mm = (mmk[:, None, None] > mmp[None, :, :]).astype(np.float32)
    ppp = (mmp > 0.5).astype(np.float32)
    mm = mm * ppp[None] + (mmk == 1.0).astype(np.float32)[:, None, None]
    mm = (mm > 0).astype(np.float32)
    z = c1 * mm * np.float32(10.0)
    z -= z.max(axis=0, keepdims=True)
    e = np.exp(z)
    return e / e.sum(axis=0, keepdims=True)

def kernel(f, b, mask):
    f = np.asarray(f, dtype=np.float32)
    b = np.asarray(b, dtype=np.float32)
    mask = np.asarray(mask, dtype=np.float32)
    B = f.shape[0]
    maskc = 1.0 - mask
    nc = _build_nc()
    res = bass_utils.run_bass_kernel_spmd(nc, _make_in_maps(f, b), list(range(8)))
    out = np.empty((B, L, 63, 63), np.float32)
    for smp in range(B):
        cos = np.empty((L, 64, 63), np.float32)
        for half in range(2):
            o = res.results[2*smp + half]["o"]            # (8, 128, 2048)
            cos[:, 32*half:32*half+32, :] = o.reshape(L, 32, 64)[:, :, 0:63]
        out[smp] = _epilogue(np.ascontiguousarray(cos[:, 0:63, :]), maskc[smp, 0])
    return out
